# revision 1
# baseline (speedup 1.0000x reference)
"""CapsuleLayer (dynamic routing) Trainium2 kernel.

Math (per example a):
  H[a,b,c,j] = sum_i x[a,c,i] * W[b,c,j,i]          (inputs_hat)
  3 routing iterations of:
    coef = softmax_b(L); s = sum_c coef*H; out = squash(s); L += sum_d out*H

Distribution: data-parallel over batch, 512 = 8 cores x 64 examples.

Per-core layout: SBUF partition p = b0*64 + a  (b0 = capsule_half, a = local
example), so the routing loop is fully partition-parallel; only the softmax
denominator needs a tiny cross-partition (p <-> p+64) fixup via SBUF DMA.

H is generated on the tensor engine with the *inputs* as the stationary
operand (M = examples -> PSUM partitions already match the routing layout).
K=8 contraction is packed 4 row-tiles (4 c values) x 2 col-tiles (b halves)
via tile_position so 8 small matmuls run concurrently in the PE array.

H is stored bf16 [128, (b16, d16, c144)]: every big routing op then runs in
DVE 2x mode (the b-update multiply keeps packing via a duplicated-pair
operand). The c axis is split DVE [0, CSPL) / Pool [CSPL, 144) sized so both
engines finish each big pass together (DVE also carries the squash chain).
The softmax denominator's cross-half (p <-> p^64) combine is one PE matmul
against a host-built pair-sum 0/1 matrix; activation tables (Sqrt/Exp) are
pre-loaded by dummy ops a phase ahead so the serial chains never pay the
~1.3us table switch.
"""

import os
import sys

for _p in ("/opt/trn_rl_repo",):
    if _p not in sys.path:
        sys.path.insert(0, _p)

from contextlib import ExitStack

import numpy as np

import concourse.bass as bass
import concourse.mybir as mybir
from concourse import tile
from concourse.bass_utils import run_bass_kernel_spmd

F32 = mybir.dt.float32
BF16 = mybir.dt.bfloat16
AF = mybir.ActivationFunctionType
ALU = mybir.AluOpType
AX = mybir.AxisListType

B = 512
NCORES = 8
BS = B // NCORES  # 64 examples per core
NCAP = 32
B16 = 16  # capsules per half
CIN = 144
CQ = 36  # c // 4
D = 16
I8 = 8
EPS = 1e-7
ROUTINGS = 3

HFREE = B16 * D * CIN  # 36864 elements per partition

CSPL = int(os.environ.get("K_CSPL", "84"))
CSPL2 = int(os.environ.get("K_CSPL2", "88"))
BSPL = int(os.environ.get("K_BSPL", "84"))
LBF16 = int(os.environ.get("K_LBF16", "1"))


def _build_program() -> bass.Bass:
    nc = bass.Bass()
    ilhs_d = nc.declare_dram_parameter("ilhs", [32, CQ * BS], BF16, isOutput=False)
    wrhs_d = nc.declare_dram_parameter("wrhs", [32, CQ * 512], BF16, isOutput=False)
    # pair-sum matrix: psmat[k, m] = 1 iff k == m or k == m ^ 64; one matmul
    # with it computes Dh[p] + Dh[p^64] for every p (the softmax denominator
    # needs both capsule halves), replacing a round-trip SBUF DMA swap
    psmat_d = nc.declare_dram_parameter("psmat", [128, 128], BF16, isOutput=False)
    out_d = nc.declare_dram_parameter("out", [BS, NCAP, D], F32, isOutput=True)

    LDT = BF16 if LBF16 else F32

    with ExitStack() as ctx:
        tc = ctx.enter_context(tile.TileContext(nc))
        cpool = ctx.enter_context(tc.tile_pool(name="const", bufs=1))

        H1a = cpool.tile([128, B16 * D * 72], BF16)  # c in [0, 72)
        H1b = cpool.tile([128, B16 * D * 72], BF16)  # c in [72, 144)
        # scratch for the big elementwise passes; allocated up front so
        # its space never aliases the W-streaming pool (an alias adds a
        # WAR dependency that delays the first routing ops until every
        # matmul has consumed its W chunk)
        prod = cpool.tile([128, HFREE], BF16)
        ilhs_t = cpool.tile([128, CQ * BS], BF16)

        # persistent small tensors
        s_t = cpool.tile([128, 256], F32)  # (b16, d)
        L_t = cpool.tile([128, B16 * CIN], LDT)  # logits (b16, c)
        Ltmp = cpool.tile([128, B16 * CIN], LDT)
        E_t = cpool.tile([128, B16 * CIN], BF16)
        C_t = cpool.tile([128, B16 * CIN], BF16)
        Dh = cpool.tile([128, CIN], BF16)
        Dt8 = cpool.tile([128, 8 * CIN], BF16)
        Rh = cpool.tile([128, CIN], BF16)
        sq = cpool.tile([128, 256], F32)
        n2 = cpool.tile([128, B16], F32)
        t1 = cpool.tile([128, B16], F32)
        r1 = cpool.tile([128, B16], F32)
        rs = cpool.tile([128, B16], F32)
        fac = cpool.tile([128, B16], F32)
        outB2 = cpool.tile([128, 512], BF16)  # squash out duplicated per c-pair
        outF = cpool.tile([128, 256], F32)
        epsb = cpool.tile([128, 1], F32)
        psmat = cpool.tile([128, 128], BF16)
        adum = cpool.tile([128, 1], F32)  # activation-table preload scratch
        adum2 = cpool.tile([128, 1], F32)
        nc.vector.memset(epsb[:], EPS)
        nc.vector.memset(adum[:], 1.0)

        # ilhs split across two queues so the r=2,3 row groups don't wait for
        # serialized issue behind r=0,1; psmat (needed only at the first
        # softmax) goes last
        for r in range(4):
            nc.scalar.dma_start(ilhs_t[32 * r : 32 * r + 8, :], ilhs_d[8 * r : 8 * r + 8, :])

        H1av = H1a[:].rearrange("p (b d c) -> p b d c", b=B16, d=D)
        H1bv = H1b[:].rearrange("p (b d c) -> p b d c", b=B16, d=D)

        def hslice(c0, c1):
            # view of H columns [c0, c1) — must not cross c=72
            if c1 <= 72:
                return H1av[:, :, :, c0:c1]
            assert c0 >= 72
            return H1bv[:, :, :, c0 - 72 : c1 - 72]

        V = nc.vector
        P = nc.gpsimd
        A = nc.scalar

        prodv = prod[:].rearrange("p (b d c) -> p b d c", b=B16, d=D)

        def pool_fold(lo, w, eng=None):
            # fold w columns starting at lo into column lo
            eng = eng or P
            while w > 1:
                half = w // 2
                eng.tensor_tensor(
                    prodv[:, :, :, lo : lo + half],
                    prodv[:, :, :, lo : lo + half],
                    prodv[:, :, :, lo + half : lo + 2 * half],
                    op=ALU.add,
                )
                if w % 2 == 1:
                    eng.tensor_tensor(
                        prodv[:, :, :, lo : lo + 1], prodv[:, :, :, lo : lo + 1],
                        prodv[:, :, :, lo + w - 1 : lo + w], op=ALU.add,
                    )
                w = half

        # ---- H generation ----
        with (
            tc.tile_pool(name="w", bufs=3) as wpool,
            tc.tile_pool(name="psum", bufs=8, space="PSUM") as pp,
        ):
            # spread the W row-group loads across two issuing engines so the
            # transfers run in parallel instead of serializing on one queue,
            # and stream in 4 chunks so matmuls start early
            dma_eng = [nc.sync, nc.sync, nc.gpsimd, nc.gpsimd]
            CHW = 9 * 512
            for h in range(4):
                wc = wpool.tile([128, CHW], BF16)
                # h=0 loads in two waves so the first matmuls unblock after a
                # third of the transfer instead of the whole chunk
                waves = [(0, 3 * 512), (3 * 512, CHW)] if h == 0 else [(0, CHW)]
                for wlo, whi in waves:
                    for r in range(4):
                        dma_eng[r].dma_start(
                            wc[32 * r : 32 * r + 8, wlo:whi],
                            wrhs_d[8 * r : 8 * r + 8, h * CHW + wlo : h * CHW + whi],
                        )
                for cql in range(9):
                    cq = h * 9 + cql
                    lhs = ilhs_t[:, cq * BS : (cq + 1) * BS]
                    if True:
                        for r in range(4):
                            c = 4 * cq + r
                            pts = pp.tile([128, 256], F32, tag="ptsm")
                            for b0 in range(2):
                                rhs = wc[
                                    32 * r : 32 * r + 8,
                                    cql * 512 + b0 * 256 : cql * 512 + b0 * 256 + 256,
                                ]
                                nc.tensor.matmul(
                                    pts[b0 * 64 : (b0 + 1) * 64, :],
                                    lhs[32 * r : 32 * r + 8, :],
                                    rhs,
                                    start=True,
                                    stop=True,
                                    tile_position=(32 * r, b0 * 64),
                                )
                            dsts = hslice(c, c + 1).squeeze(3)
                            srcs = pts[:].rearrange("p (b d) -> p b d", b=B16)
                            if r % 2 == 0:
                                nc.vector.tensor_copy(dsts, srcs)
                            else:
                                nc.scalar.copy(dsts, srcs)
        # psmat is needed only at the first softmax (~70us in): issue it
        # after the drain copies so Act's H-phase work starts sooner
        nc.scalar.dma_start(psmat[:], psmat_d[:, :])

        # ---- routing ----
        if True:
            s_v = s_t[:].rearrange("p (b d) -> p b d", b=B16)
            L_v = L_t[:].rearrange("p (b c) -> p b c", b=B16)
            Lt_v = Ltmp[:].rearrange("p (b c) -> p b c", b=B16)
            E_v = E_t[:].rearrange("p (b c) -> p b c", b=B16)
            C_v = C_t[:].rearrange("p (b c) -> p b c", b=B16)

            def s0_phase():
                # uniform coefficients: s0 = (1/32) sum_c H, as 8 fine chunk
                # trees pipelining with the tail of the H-drain copies; Pool
                # (idle during H-gen) takes the early 5 chunks
                for k in range(8):
                    lo = 18 * k
                    eng = P if k < 5 else V
                    eng.tensor_tensor(
                        prodv[:, :, :, lo : lo + 9],
                        hslice(lo, lo + 9),
                        hslice(lo + 9, lo + 18),
                        op=ALU.add,
                    )
                    pool_fold(lo, 9, eng=eng)
                for eng, dst, srcc in (
                    (V, 0, 18), (P, 72, 90), (V, 36, 54), (P, 108, 126),
                    (V, 0, 36), (P, 72, 108), (V, 0, 72),
                ):
                    eng.tensor_tensor(
                        prodv[:, :, :, dst : dst + 1], prodv[:, :, :, dst : dst + 1],
                        prodv[:, :, :, srcc : srcc + 1], op=ALU.add,
                    )
                V.tensor_scalar(
                    s_v, prodv[:, :, :, 0:1].squeeze(3), 1.0 / NCAP, None,
                    op0=ALU.mult,
                )

            def vtree(lo, w):
                # DVE: halve [lo, lo+w) until w <= 10 (odd spills fold into
                # col lo), then one strided reduce finishes it into s_v.
                while w > 10:
                    if w % 2 == 1:
                        V.tensor_tensor(
                            prodv[:, :, :, lo : lo + 1], prodv[:, :, :, lo : lo + 1],
                            prodv[:, :, :, lo + w - 1 : lo + w], op=ALU.add,
                        )
                        w -= 1
                    w //= 2
                    V.tensor_tensor(
                        prodv[:, :, :, lo : lo + w],
                        prodv[:, :, :, lo : lo + w],
                        prodv[:, :, :, lo + w : lo + 2 * w],
                        op=ALU.add,
                    )
                while w > 2:
                    if w % 2 == 1:
                        V.tensor_tensor(
                            prodv[:, :, :, lo : lo + 1], prodv[:, :, :, lo : lo + 1],
                            prodv[:, :, :, lo + w - 1 : lo + w], op=ALU.add,
                        )
                        w -= 1
                    w //= 2
                    V.tensor_tensor(
                        prodv[:, :, :, lo : lo + w],
                        prodv[:, :, :, lo : lo + w],
                        prodv[:, :, :, lo + w : lo + 2 * w],
                        op=ALU.add,
                    )
                # final pair-add writes s_v directly (skips a copy)
                V.tensor_tensor(
                    s_v, prodv[:, :, :, lo : lo + 1].squeeze(3),
                    prodv[:, :, :, lo + 1 : lo + 2].squeeze(3), op=ALU.add,
                )

            def s_phase(cs=CSPL):
                # Pool range first: Pool is the scarcer engine in steady state
                cb = C_v.unsqueeze(2).broadcast_to((128, B16, D, CIN))
                P.tensor_tensor(
                    prodv[:, :, :, cs:144], hslice(cs, 144),
                    cb[:, :, :, cs:144], op=ALU.mult,
                )
                pool_fold(cs, 144 - cs)
                V.tensor_tensor(
                    prodv[:, :, :, 0:72], hslice(0, 72),
                    cb[:, :, :, 0:72], op=ALU.mult,
                )
                V.tensor_tensor(
                    prodv[:, :, :, 72:cs], hslice(72, cs),
                    cb[:, :, :, 72:cs], op=ALU.mult,
                )
                vtree(0, cs)
                V.tensor_tensor(
                    s_v, s_v, prodv[:, :, :, cs : cs + 1].squeeze(3), op=ALU.add
                )

            outB2v = outB2[:].rearrange("p (bd c2) -> p bd c2", c2=2)
            outBa = outB2v[:, :, 0:1].squeeze(2).rearrange("p (b d) -> p b d", b=B16)
            outBb = outB2v[:, :, 1:2].squeeze(2).rearrange("p (b d) -> p b d", b=B16)

            def squash(final: bool):
                nc.vector.tensor_tensor(sq[:], s_t[:], s_t[:], op=ALU.mult)
                nc.vector.reduce_sum(
                    n2[:], sq[:].rearrange("p (b d) -> p b d", b=B16), axis=AX.X
                )
                nc.scalar.activation(rs[:], n2[:], AF.Sqrt, bias=epsb[:])
                # t1 = (n2 + 1) * sqrt(n2 + eps) in one fused op
                nc.vector.scalar_tensor_tensor(
                    t1[:], n2[:], 1.0, rs[:], op0=ALU.add, op1=ALU.mult
                )
                nc.vector.reciprocal(r1[:], t1[:])
                nc.vector.tensor_tensor(fac[:], n2[:], r1[:], op=ALU.mult)
                facb = fac[:].unsqueeze(2).broadcast_to((128, B16, D))
                if final:
                    nc.vector.tensor_tensor(outF[:].rearrange("p (b d) -> p b d", b=B16), s_v, facb, op=ALU.mult)
                else:
                    # write the squashed output directly in duplicated-pair
                    # form, one copy per engine (Pool is idle during squash
                    # and its own b_update multiply queues behind its copy)
                    nc.vector.tensor_tensor(outBa, s_v, facb, op=ALU.mult)
                    P.tensor_tensor(outBb, s_v, facb, op=ALU.mult)

            H1abd = H1a[:].rearrange("p (bd c) -> p bd c", c=72)
            H1bbd = H1b[:].rearrange("p (bd c) -> p bd c", c=72)
            prodbd = prod[:].rearrange("p (bd c) -> p bd c", c=CIN)

            def b_update(first: bool):
                # Pool range first, then DVE. The multiplies split at the
                # c=72 H-tile boundary, but the d-trees and logit adds only
                # touch prod/L (contiguous), so DVE runs those as single
                # full-range ops. The split sits DVE-ward of the softmax's
                # (BSPL > CSPL is safe: DVE finishes its logit writes before
                # Pool, so the exp's cross-range read never stalls).
                mults = [(P, BSPL, 144), (V, 0, 72), (V, 72, BSPL)]
                for eng, lo, hi in mults:
                    npair = (hi - lo) // 2
                    hb = H1abd if hi <= 72 else H1bbd
                    ho = lo if hi <= 72 else lo - 72
                    h_in = hb[:, :, ho : ho + (hi - lo)].rearrange(
                        "p bd (cp c2) -> p bd cp c2", c2=2
                    )
                    o_in = outB2v.unsqueeze(2).broadcast_to((128, 256, npair, 2))
                    p_out = prodbd[:, :, lo:hi].rearrange(
                        "p bd (cp c2) -> p bd cp c2", c2=2
                    )
                    eng.tensor_tensor(p_out, h_in, o_in, op=ALU.mult)
                for eng, lo, hi in ((P, BSPL, 144), (V, 0, BSPL)):
                    for w in (8, 4, 2):
                        eng.tensor_tensor(
                            prodv[:, :, 0:w, lo:hi],
                            prodv[:, :, 0:w, lo:hi],
                            prodv[:, :, w : 2 * w, lo:hi],
                            op=ALU.add,
                        )
                    d0 = prodv[:, :, 0:1, lo:hi].squeeze(2)
                    d1 = prodv[:, :, 1:2, lo:hi].squeeze(2)
                    if first:
                        eng.tensor_tensor(L_v[:, :, lo:hi], d0, d1, op=ALU.add)
                    else:
                        eng.tensor_tensor(Lt_v[:, :, lo:hi], d0, d1, op=ALU.add)
                        eng.tensor_tensor(
                            L_v[:, :, lo:hi], L_v[:, :, lo:hi],
                            Lt_v[:, :, lo:hi], op=ALU.add,
                        )

            def softmax(rpp, v_first=False):
                # c-range split: Pool's range first so its exp/coefficients
                # are ready when the next s_phase's pool multiply starts; the
                # two denominator trees run on different engines in parallel.
                Dt8v = Dt8[:].rearrange("p (b c) -> p b c", b=8)
                order = ((0, CSPL, V), (CSPL, CIN, P)) if v_first else ((CSPL, CIN, P), (0, CSPL, V))
                for lo, hi, teng in order:
                    nc.scalar.activation(
                        E_v[:, :, lo:hi], L_v[:, :, lo:hi], AF.Exp
                    )
                    teng.tensor_tensor(
                        Dt8v[:, 0:8, lo:hi], E_v[:, 0:8, lo:hi],
                        E_v[:, 8:16, lo:hi], op=ALU.add,
                    )
                    teng.tensor_tensor(
                        Dt8v[:, 0:4, lo:hi], Dt8v[:, 0:4, lo:hi],
                        Dt8v[:, 4:8, lo:hi], op=ALU.add,
                    )
                    teng.tensor_tensor(
                        Dt8v[:, 0:2, lo:hi], Dt8v[:, 0:2, lo:hi],
                        Dt8v[:, 2:4, lo:hi], op=ALU.add,
                    )
                    teng.tensor_tensor(
                        Dh[:, lo:hi].unsqueeze(1), Dt8v[:, 0:1, lo:hi],
                        Dt8v[:, 1:2, lo:hi], op=ALU.add,
                    )
                # combine the capsule halves with a pair-sum matmul per
                # c-range: dps[p] = Dh[p] + Dh[p^64] for every partition.
                # Separate matmul + reciprocal per range keeps the Pool-range
                # softmax chain independent of the (later) DVE-range one.
                rb = Rh[:].unsqueeze(1).broadcast_to((128, B16, CIN))
                for lo, hi, teng in order:
                    # full-bank tile per range: a matmul's start=True clears
                    # its whole bank, so the two ranges must not share one
                    dps = rpp.tile([128, 512], F32)
                    w = hi - lo
                    nc.tensor.matmul(
                        dps[:, 0:w], psmat[:], Dh[:, lo:hi],
                        start=True, stop=True,
                    )
                    with nc.allow_low_precision(
                        reason="softmax coefficients are bf16 throughout"
                    ):
                        nc.vector.reciprocal(Rh[:, lo:hi], dps[:, 0:w])
                    teng.tensor_tensor(
                        C_v[:, :, lo:hi], E_v[:, :, lo:hi], rb[:, :, lo:hi],
                        op=ALU.mult,
                    )
                # pre-load the Sqrt table while the next s_phase runs
                nc.scalar.activation(adum2[:], adum[:], AF.Sqrt)

            with tc.tile_pool(name="rpsum", bufs=2, space="PSUM") as rpp:
                # pre-load the Sqrt activation table under the H-drain tail
                nc.scalar.activation(adum2[:], adum[:], AF.Sqrt)
                for it in range(ROUTINGS):
                    if it == 0:
                        s0_phase()
                    else:
                        # final iteration: DVE still has the squash + output
                        # tail after its tree, so Pool takes a bigger share
                        s_phase(CSPL if it < ROUTINGS - 1 else CSPL2)
                    squash(final=(it == ROUTINGS - 1))
                    if it < ROUTINGS - 1:
                        # pre-load the Exp table while b_update runs
                        nc.scalar.activation(adum2[:], adum[:], AF.Exp)
                        b_update(first=(it == 0))
                        # after the second softmax the DVE chain (final
                        # s-phase + squash + output) is critical while Pool
                        # has end slack: DVE's range resolves first there
                        softmax(rpp, v_first=(it == 1))

            for b0 in range(2):
                oap = out_d[:, b0 * B16 : (b0 + 1) * B16, :].rearrange(
                    "a b d -> a (b d)"
                )
                nc.sync.dma_start(oap, outF[b0 * 64 : (b0 + 1) * 64, :])

    # The TRN2 matmul ISA encoding only fits one sync wait; Tile can emit
    # several. Run the bacc fix-up passes: excess matmul waits move to the
    # paired ldweights, and any instruction still holding >1 wait gets them
    # split into preceding EventSemaphore instructions.
    import bass_rust as _bass_rust

    _bass_rust.move_matmul_waits_to_ldweights(nc.m)
    _bass_rust.generate_event_semaphores(nc)
    return nc


def _bf16(x: np.ndarray) -> np.ndarray:
    import ml_dtypes

    return x.astype(ml_dtypes.bfloat16)


def _pack_w(W: np.ndarray) -> np.ndarray:
    # wrhs[8r+i, cq*512 + b*16 + j] = W[b, 4cq+r, j, i]
    wrhs = np.empty((32, CQ * 512), np.float32)
    for r in range(4):
        blk = W[:, r::4, :, :]  # [b, cq, j, i]
        wrhs[8 * r : 8 * r + 8, :] = np.ascontiguousarray(
            blk.transpose(3, 1, 0, 2)
        ).reshape(8, CQ * 512)
    return _bf16(wrhs)


def _pack_x(xs: np.ndarray) -> np.ndarray:
    # ilhs[8r+i, cq*64 + a] = xs[a, 4cq+r, i]
    ilhs = np.empty((32, CQ * BS), np.float32)
    for r in range(4):
        blk = xs[:, r::4, :]  # [a, cq, i]
        ilhs[8 * r : 8 * r + 8, :] = np.ascontiguousarray(blk.transpose(2, 1, 0)).reshape(
            8, CQ * BS
        )
    return _bf16(ilhs)


_CACHED = {}


def _get_program():
    if "nc" not in _CACHED:
        _CACHED["nc"] = _build_program()
    return _CACHED["nc"]


def _psmat() -> np.ndarray:
    m = np.zeros((128, 128), np.float32)
    for k in range(128):
        m[k, k] = 1.0
        m[k, k ^ 64] = 1.0
    return _bf16(m)


def kernel(inputs: np.ndarray, W: np.ndarray) -> np.ndarray:
    inputs = np.asarray(inputs, np.float32)
    W = np.asarray(W, np.float32)
    nc = _get_program()
    wrhs = _pack_w(W)
    psm = _psmat()
    in_maps = []
    for k in range(NCORES):
        xs = inputs[k * BS : (k + 1) * BS]
        in_maps.append({"ilhs": _pack_x(xs), "wrhs": wrhs, "psmat": psm})
    res = run_bass_kernel_spmd(nc, in_maps, core_ids=list(range(NCORES)))
    out = np.concatenate([res.results[k]["out"] for k in range(NCORES)], axis=0)
    return out

